# revision 6
# baseline (speedup 1.0000x reference)
"""Trainium2 Bass kernel for the anchor-based NMS matcher.

Math (see problem reference): per (batch b, organ o), over Qp=8192 anchor
queries q:
    cost_class = -sigmoid(logit)
    cost_bbox  = sum_d |anchor_d - tgt_d|            (cxcyczwhd space)
    cost_giou  = -giou3d(xyzxyz(clip(anchor,0)), xyzxyz(tgt))
    C = 5*cb + 2*cc + 2*cg
    matches     = one_hot(argmin_q C) * present
    soft_labels = present ? clip((cg-cgmax)/(cgmin-cgmax), 0) : -1

Device strategy (8 cores, data-parallel over batch, 2 batch items/core):
  SBUF layout: 120 partitions = (b_local 2) x (organ 20) x (q-chunk 3),
  free dim N=2752 (3*2752=8256, q padded 8192->8256 with edge dup).
  All per-(b,o) target quantities become per-partition scalars, enabling
  fused tensor_scalar / scalar_tensor_tensor / activation(bias,scale) ops.
  Anchor-derived planes (clipped lt/rb/size/vol) are precomputed on host and
  DMA'd into one interleaved SBUF tile; big passes are spread across
  DVE / ACT / GPSIMD.  giou needs one reciprocal via
      -giou + 1 = 1 - (u^2 + inter*vol_c)/(u*vol_c),  u = union
  ranking with negC = sig - 2.5*cb + frac (argmax negC == argmin C), and soft
  labels normalized in frac-space (affine-invariant).
  Per-partition argmax via DVE max/max_index; the 3 q-chunks per (b,o) are
  combined through tiny PE transposes ([120,1] -> [1,120]) so cross-chunk
  logic runs on partition-0 row vectors.  The winning q index per (b,o) is
  DMA'd out as a 40-element row and scattered into the one-hot on host.
"""

import numpy as np

import concourse.bacc as bacc
import concourse.bass as bass
import concourse.mybir as mybir
from concourse.bass_utils import run_bass_kernel_spmd
from concourse.masks import make_identity
from concourse.tile import TileContext

F32 = mybir.dt.float32
ALU = mybir.AluOpType
ACTF = mybir.ActivationFunctionType
AXL = mybir.AxisListType

BS, O, QP = 16, 20, 8192
NCORES = 8
BL = BS // NCORES        # batch items per core
NCH = 3                  # q chunks per organ
N = 2752                 # chunk width; 3*2752 = 8256 = 8192 + 64 pad
NPAIR = BL * O           # 40 (b,o) pairs per core
P = NPAIR * NCH          # 120 partitions
NPLANES = 16             # alt0-2, arb0-2, rs0-2, vola, a0-5

_BUILT = {}


def _build_nc():
    nc = bacc.Bacc("TRN2", target_bir_lowering=False, debug=False)
    ath = nc.dram_tensor("ath", [NPLANES, 60, N], F32, kind="ExternalInput")
    lg = nc.dram_tensor("lg", [P, N], F32, kind="ExternalInput")
    sc = nc.dram_tensor("sc", [P, 20], F32, kind="ExternalInput")
    rw = nc.dram_tensor("rw", [1, 256], F32, kind="ExternalInput")
    sout = nc.dram_tensor("sout", [P, N], F32, kind="ExternalOutput")
    awout = nc.dram_tensor("awout", [1, NPAIR], F32, kind="ExternalOutput")

    with TileContext(nc) as tc:
        with (
            tc.tile_pool(name="big", bufs=1) as big,
            tc.tile_pool(name="sm", bufs=1) as sm,
            tc.tile_pool(name="ps", bufs=1, space="PSUM") as ps,
        ):
            # ---------------- small/const tiles ----------------
            sct = sm.tile([P, 20], F32, tag="sct")
            nc.sync.dma_start(out=sct[:], in_=sc[:])
            rwt = sm.tile([1, 256], F32, tag="rwt")
            nc.sync.dma_start(out=rwt[:], in_=rw[:])
            ident = sm.tile([120, 120], F32, tag="ident")
            make_identity(nc, ident[:])
            ones11 = sm.tile([1, 1], F32, tag="ones11")
            nc.vector.memset(ones11[:], 1.0)

            def col(i):  # per-partition scalar column
                return sct[:, i : i + 1]

            NT = [col(d) for d in range(6)]       # -t_d
            BLT = [col(6 + d) for d in range(3)]
            BRB = [col(9 + d) for d in range(3)]
            FD = [col(12 + d) for d in range(3)]
            VOLB = col(15)
            QOFF = col(16)
            FLOOR = col(17)
            prs_row = rwt[:, 0:120]
            prsm1_row = rwt[:, 120:240]

            # ---------------- big input tile ----------------
            ain = big.tile([P, NPLANES, N], F32, tag="ain")

            def v(j):
                return ain[:, j, :]

            def load_group(j0, j1):
                src = ath[j0:j1].rearrange("i p n -> p i n")
                nc.sync.dma_start(out=ain[0:60, j0:j1, :], in_=src)
                nc.sync.dma_start(out=ain[60:120, j0:j1, :], in_=src)

            lgt = big.tile([P, N], F32, tag="lg")
            nc.sync.dma_start(out=lgt[:], in_=lg[:])

            load_group(0, 6)      # alt0-2, arb0-2
            alt = [v(d) for d in range(3)]
            arb = [v(3 + d) for d in range(3)]

            # mx_d = max(alt_d, Blt_d), in place
            for d in range(3):
                nc.vector.tensor_scalar_max(out=alt[d], in0=alt[d], scalar1=BLT[d])
            mx = alt
            # m_d = min(arb_d, Brb_d) - mx_d, in place
            for d in range(3):
                nc.vector.scalar_tensor_tensor(
                    out=arb[d], in0=arb[d], scalar=BRB[d], in1=mx[d],
                    op0=ALU.min, op1=ALU.subtract)
            m = arb

            load_group(6, 10)     # rs0-2, vola
            rs = [v(6 + d) for d in range(3)]
            vola = v(9)

            # vc_d = (rs_d + f_d) - m_d, in place over rs (before relu of m!)
            for d in range(3):
                nc.vector.scalar_tensor_tensor(
                    out=rs[d], in0=rs[d], scalar=FD[d], in1=m[d],
                    op0=ALU.add, op1=ALU.subtract)
            vc = rs
            # r_d = relu(m_d) in place (ACT)
            for d in range(3):
                nc.scalar.activation(m[d], m[d], ACTF.Relu)
            # inter = r0*r1*r2 into v(3)
            nc.gpsimd.tensor_tensor(out=v(3), in0=v(3), in1=v(4), op=ALU.mult)
            nc.gpsimd.tensor_tensor(out=v(3), in0=v(3), in1=v(5), op=ALU.mult)
            inter = v(3)
            # union = (vola + volb) - inter, in place over vola
            nc.vector.scalar_tensor_tensor(
                out=vola, in0=vola, scalar=VOLB, in1=inter,
                op0=ALU.add, op1=ALU.subtract)
            union = vola
            # u2 = union^2 -> v(0)
            nc.scalar.activation(v(0), union, ACTF.Square)
            # volc = vc0*vc1*vc2 -> v(1)
            nc.gpsimd.tensor_tensor(out=v(1), in0=vc[0], in1=vc[1], op=ALU.mult)
            nc.gpsimd.tensor_tensor(out=v(1), in0=v(1), in1=vc[2], op=ALU.mult)
            volc = v(1)
            # den = union * volc -> v(2)
            nc.gpsimd.tensor_tensor(out=v(2), in0=union, in1=volc, op=ALU.mult)
            den = v(2)
            # ivc = inter * volc, in place over inter (after union consumed it)
            nc.gpsimd.tensor_tensor(out=inter, in0=inter, in1=volc, op=ALU.mult)
            ivc = inter
            # num = u2 + ivc, in place over u2
            nc.vector.tensor_tensor(out=v(0), in0=v(0), in1=ivc, op=ALU.add)
            # rden ~ 1/den -> v(4)
            nc.vector.reciprocal_approx_fast(out=v(4), in_=den)
            # frac = num * rden, in place -> v(0)
            nc.vector.tensor_tensor(out=v(0), in0=v(0), in1=v(4), op=ALU.mult)
            frac = v(0)

            load_group(10, 16)    # a0-5
            a = [v(10 + d) for d in range(6)]

            # sigmoid (in place over logits)
            nc.scalar.activation(lgt[:], lgt[:], ACTF.Sigmoid)
            sig = lgt[:]
            # ab_d = |a_d - t_d| in place (ACT)
            for d in range(6):
                nc.scalar.activation(a[d], a[d], ACTF.Abs, bias=NT[d], scale=1.0)
            # cb tree: GP pairs, DVE joins -> v(10)
            nc.gpsimd.tensor_tensor(out=v(10), in0=v(10), in1=v(11), op=ALU.add)
            nc.gpsimd.tensor_tensor(out=v(12), in0=v(12), in1=v(13), op=ALU.add)
            nc.gpsimd.tensor_tensor(out=v(14), in0=v(14), in1=v(15), op=ALU.add)
            nc.vector.tensor_tensor(out=v(10), in0=v(10), in1=v(12), op=ALU.add)
            nc.vector.tensor_tensor(out=v(10), in0=v(10), in1=v(14), op=ALU.add)
            cb = v(10)
            # u1 = (cb * -2.5 + sig); negc = u1 + frac (in place over lgt)
            nc.vector.scalar_tensor_tensor(
                out=sig, in0=cb, scalar=-2.5, in1=sig, op0=ALU.mult, op1=ALU.add)
            nc.vector.tensor_tensor(out=sig, in0=sig, in1=frac, op=ALU.add)
            negc = sig

            # per-partition top-8 + index, frac min/max
            mx8 = sm.tile([P, 8], F32, tag="mx8")
            ix8 = sm.tile([P, 8], mybir.dt.uint32, tag="ix8")
            nc.vector.max(out=mx8[:], in_=negc)
            nc.vector.max_index(out=ix8[:], in_max=mx8[:], in_values=negc)
            fmx = sm.tile([P, 1], F32, tag="fmx")
            fmn = sm.tile([P, 1], F32, tag="fmn")
            nc.vector.tensor_reduce(out=fmx[:], in_=frac, axis=AXL.X, op=ALU.max)
            nc.vector.tensor_reduce(out=fmn[:], in_=frac, axis=AXL.X, op=ALU.min)

            # ---------------- cross-chunk combine (tiny) ----------------
            ixf = sm.tile([P, 1], F32, tag="ixf")
            nc.vector.tensor_copy(out=ixf[:], in_=ix8[:, 0:1])
            gidx = sm.tile([P, 1], F32, tag="gidx")
            nc.vector.tensor_scalar_add(out=gidx[:], in0=ixf[:], scalar1=QOFF)

            def to_row(colap, tag):  # [120,1] sbuf -> [1,120] psum
                r = ps.tile([1, 120], F32, tag=tag, name=tag)
                nc.tensor.transpose(r[:], colap, ident[:])
                return r

            mx_t = to_row(mx8[:, 0:1], "mx_t")
            gx_t = to_row(gidx[:], "gx_t")
            fx_t = to_row(fmx[:], "fx_t")
            fn_t = to_row(fmn[:], "fn_t")

            def g3(ap120):  # [1,120] -> [1,40,3]
                return ap120.rearrange("p (g c) -> p g c", c=3)

            def b3(ap40):  # [1,40] -> [1,40,3] broadcast read
                return ap40[:, :, None].broadcast_to((1, 40, 3))

            def srow(tag, w=120):
                return sm.tile([1, w], F32, tag=tag, name=tag)

            # group max of chunk maxima; first-winner select of global q
            gmax = srow("gmax", 40)
            nc.vector.tensor_reduce(out=gmax[:], in_=g3(mx_t[:]), axis=AXL.X, op=ALU.max)
            eqm = srow("eqm")
            nc.vector.tensor_tensor(out=g3(eqm[:]), in0=g3(mx_t[:]), in1=b3(gmax[:]),
                                    op=ALU.is_equal)
            eq = g3(eqm[:])
            ne0 = srow("ne0", 40)
            ne1 = srow("ne1", 40)
            nc.vector.tensor_scalar(out=ne0[:], in0=eq[:, :, 0], scalar1=-1.0,
                                    scalar2=1.0, op0=ALU.mult, op1=ALU.add)
            nc.vector.tensor_scalar(out=ne1[:], in0=eq[:, :, 1], scalar1=-1.0,
                                    scalar2=1.0, op0=ALU.mult, op1=ALU.add)
            w1 = srow("w1", 40)
            nc.vector.tensor_tensor(out=w1[:], in0=eq[:, :, 1], in1=ne0[:], op=ALU.mult)
            w2 = srow("w2", 40)
            nc.vector.tensor_tensor(out=w2[:], in0=eq[:, :, 2], in1=ne0[:], op=ALU.mult)
            nc.vector.tensor_tensor(out=w2[:], in0=w2[:], in1=ne1[:], op=ALU.mult)
            gx = g3(gx_t[:])
            aw = srow("aw", 40)
            t1r = srow("t1r", 40)
            t2r = srow("t2r", 40)
            nc.vector.tensor_tensor(out=aw[:], in0=eq[:, :, 0], in1=gx[:, :, 0], op=ALU.mult)
            nc.vector.tensor_tensor(out=t1r[:], in0=w1[:], in1=gx[:, :, 1], op=ALU.mult)
            nc.vector.tensor_tensor(out=t2r[:], in0=w2[:], in1=gx[:, :, 2], op=ALU.mult)
            nc.vector.tensor_tensor(out=aw[:], in0=aw[:], in1=t1r[:], op=ALU.add)
            nc.vector.tensor_tensor(out=aw[:], in0=aw[:], in1=t2r[:], op=ALU.add)
            nc.sync.dma_start(out=awout[:], in_=aw[:])

            # soft-label scale/bias rows (present-masked)
            gfx = srow("gfx", 40)
            gfn = srow("gfn", 40)
            nc.vector.tensor_reduce(out=gfx[:], in_=g3(fx_t[:]), axis=AXL.X, op=ALU.max)
            nc.vector.tensor_reduce(out=gfn[:], in_=g3(fn_t[:]), axis=AXL.X, op=ALU.min)
            dd = srow("dd", 40)
            nc.vector.tensor_tensor(out=dd[:], in0=gfx[:], in1=gfn[:], op=ALU.subtract)
            inv = srow("inv", 40)
            nc.vector.reciprocal(out=inv[:], in_=dd[:])
            nb = srow("nb", 40)
            nc.vector.tensor_tensor(out=nb[:], in0=gfn[:], in1=inv[:], op=ALU.mult)
            nc.vector.tensor_scalar_mul(out=nb[:], in0=nb[:], scalar1=-1.0)
            # scale_r = inv (bcast) * prs ; bias_r = nb (bcast) * prs + (prs-1)
            scale_r = srow("scale_r")
            nc.vector.tensor_tensor(out=g3(scale_r[:]), in0=b3(inv[:]),
                                    in1=g3(prs_row), op=ALU.mult)
            bias_r = srow("bias_r")
            nc.vector.tensor_tensor(out=g3(bias_r[:]), in0=b3(nb[:]),
                                    in1=g3(prs_row), op=ALU.mult)
            nc.vector.tensor_tensor(out=bias_r[:], in0=bias_r[:], in1=prsm1_row,
                                    op=ALU.add)

            def to_col(rowap, tag):  # [1,120] sbuf -> [120,1] psum
                c = ps.tile([120, 1], F32, tag=tag, name=tag)
                nc.tensor.transpose(c[:], rowap, ones11[:])
                return c

            csc = to_col(scale_r[:], "csc")
            cbi = to_col(bias_r[:], "cbi")
            scale_c = sm.tile([P, 1], F32, tag="scale_c")
            nc.vector.tensor_copy(out=scale_c[:], in_=csc[:])
            bias_c = sm.tile([P, 1], F32, tag="bias_c")
            nc.vector.tensor_copy(out=bias_c[:], in_=cbi[:])

            # ---------------- soft labels out ----------------
            nc.scalar.activation(v(5), frac, ACTF.Identity,
                                 bias=bias_c[:], scale=scale_c[:])
            nc.vector.tensor_scalar_max(out=v(5), in0=v(5), scalar1=FLOOR)
            nc.sync.dma_start(out=sout[:], in_=v(5))

    nc.finalize()
    return nc


def _prep_host(pred_logits, anchors, target_boxes, target_present):
    f32 = np.float32
    A = np.ascontiguousarray(anchors.reshape(O, QP, 6).astype(f32, copy=False))
    pad = lambda x: np.pad(x, ((0, 0), (0, NCH * N - QP)), mode="edge")

    comp = [pad(A[:, :, d]) for d in range(6)]  # [20, 8256] each
    rc = [np.maximum(comp[d], f32(0)) for d in range(3)]
    rsz = [np.maximum(comp[3 + d], f32(0)) for d in range(3)]
    alt = [rc[d] - f32(0.5) * rsz[d] for d in range(3)]
    arb = [rc[d] + f32(0.5) * rsz[d] for d in range(3)]
    vola = (rsz[0] * rsz[1]) * rsz[2]
    planes = alt + arb + rsz + [vola] + comp
    ath = np.stack([p.reshape(O, NCH, N).reshape(60, N) for p in planes])
    ath = np.ascontiguousarray(ath, dtype=f32)

    lgs = pred_logits.reshape(BS, O, QP).astype(f32, copy=False)
    lgs = np.pad(lgs, ((0, 0), (0, 0), (0, NCH * N - QP)), mode="edge")
    lg_all = lgs.reshape(BS, O, NCH, N)

    t = target_boxes.astype(f32, copy=False)          # [BS, O, 6]
    tc_, ts_ = t[..., :3], t[..., 3:]
    blt = tc_ - f32(0.5) * ts_
    brb = tc_ + f32(0.5) * ts_
    fd = brb - blt
    volb = (fd[..., 0] * fd[..., 1]) * fd[..., 2]
    prs = target_present.astype(f32, copy=False)      # [BS, O]

    in_maps = []
    for c in range(NCORES):
        b0 = c * BL
        lgc = np.ascontiguousarray(lg_all[b0 : b0 + BL].reshape(P, N), dtype=f32)
        scv = np.zeros((P, 20), f32)
        sc3 = scv.reshape(BL * O, NCH, 20)
        tb = t[b0 : b0 + BL].reshape(BL * O, 6)
        sc3[:, :, 0:6] = -tb[:, None, :]
        sc3[:, :, 6:9] = blt[b0 : b0 + BL].reshape(-1, 3)[:, None, :]
        sc3[:, :, 9:12] = brb[b0 : b0 + BL].reshape(-1, 3)[:, None, :]
        sc3[:, :, 12:15] = fd[b0 : b0 + BL].reshape(-1, 3)[:, None, :]
        sc3[:, :, 15] = volb[b0 : b0 + BL].reshape(-1)[:, None]
        sc3[:, :, 16] = np.arange(NCH, dtype=f32)[None, :] * f32(N)
        pr = prs[b0 : b0 + BL].reshape(-1)
        sc3[:, :, 17] = pr[:, None] - f32(1)  # floor: 0 if present else -1
        rwv = np.zeros((1, 256), f32)
        pr3 = np.repeat(pr, NCH)
        rwv[0, 0:P] = pr3
        rwv[0, 120 : 120 + P] = pr3 - f32(1)
        in_maps.append({"ath": ath, "lg": lgc, "sc": scv, "rw": rwv})
    return in_maps


def kernel(pred_logits, pred_boxes, anchors, target_boxes, target_present,
           num_top_queries):
    k = int(num_top_queries)
    assert k == 1, f"kernel specialized for num_top_queries=1, got {k}"

    if "nc" not in _BUILT:
        _BUILT["nc"] = _build_nc()
    nc = _BUILT["nc"]

    pred_logits = np.asarray(pred_logits)
    anchors = np.asarray(anchors)
    target_boxes = np.asarray(target_boxes)
    target_present = np.asarray(target_present)
    in_maps = _prep_host(pred_logits, anchors, target_boxes, target_present)
    res = run_bass_kernel_spmd(nc, in_maps, core_ids=list(range(NCORES)))

    matches = np.zeros((BS, O, QP), np.int32)
    soft = np.empty((BS, O, QP), np.float32)
    present = target_present.astype(bool)
    for c, r in enumerate(res.results):
        b0 = c * BL
        soft[b0 : b0 + BL] = r["sout"].reshape(BL, O, NCH * N)[:, :, :QP]
        aw = r["awout"][0].astype(np.int64)  # [40] winning q per (b_local, o)
        for pair in range(BL * O):
            b, o = b0 + pair // O, pair % O
            if present[b, o]:
                matches[b, o, aw[pair]] = 1
    return matches, soft


# revision 8
# speedup vs baseline: 1.2119x; 1.2119x over previous
"""Trainium2 Bass kernel for the anchor-based NMS matcher.

Math (see problem reference): per (batch b, organ o), over Qp=8192 anchor
queries q:
    cost_class = -sigmoid(logit)
    cost_bbox  = sum_d |anchor_d - tgt_d|            (cxcyczwhd space)
    cost_giou  = -giou3d(xyzxyz(clip(anchor,0)), xyzxyz(tgt))
    C = 5*cb + 2*cc + 2*cg
    matches     = one_hot(argmin_q C) * present
    soft_labels = present ? clip((cg-cgmax)/(cgmin-cgmax), 0) : -1

Device strategy (8 cores, data-parallel over batch, 2 batch items/core):
  SBUF layout: 120 partitions = (organ 20) x (q-chunk 6), free dim
  N=1366 (6*1366=8196, q padded 8192->8196 with edge dup).  The two batch
  items run as two interleaved half-width pass sets over the SAME anchor
  planes (loaded once -- no batch duplication of the big inputs).
  All per-(b,o) target quantities are per-partition scalars, enabling fused
  tensor_scalar / scalar_tensor_tensor / activation(bias,scale) ops.
  Anchor-derived planes (clipped lt/rb/size/vol) are precomputed on host.
  giou needs one reciprocal via
      -giou + 1 = 1 - (u^2 + inter*vol_c)/(u*vol_c),  u = union
  ranking with negC = sig - 2.5*cb + frac (argmax negC == argmin C); soft
  labels are normalized in frac-space (affine-invariant).
  Per-partition argmax via DVE max/max_index.  Per-chunk winner (value,
  global q) columns are DMA'd out and the 6-chunk combine + one-hot scatter
  happen on host (40 rows/core).  The soft-label scale/bias columns are
  produced on device via tiny PE transposes ([120,1] <-> [1,120]) so the
  cross-chunk stats logic runs on partition-0 row vectors.
"""

import numpy as np

import concourse.bacc as bacc
import concourse.bass as bass
import concourse.mybir as mybir
from concourse.bass_utils import run_bass_kernel_spmd
from concourse.masks import make_identity
from concourse.tile import TileContext

F32 = mybir.dt.float32
ALU = mybir.AluOpType
ACTF = mybir.ActivationFunctionType
AXL = mybir.AxisListType

BS, O, QP = 16, 20, 8192
NCORES = 8
BL = BS // NCORES        # batch items per core
NCH = 6                  # q chunks per organ
N = 1366                 # chunk width; 6*1366 = 8196 = 8192 + 4 pad
P = O * NCH              # 120 partitions
NPLANES = 16             # alt0-2, arb0-2, rs0-2, vola, a0-5

_BUILT = {}


def _build_nc():
    nc = bacc.Bacc("TRN2", target_bir_lowering=False, debug=False)
    ath = nc.dram_tensor("ath", [NPLANES, P, N], F32, kind="ExternalInput")
    lg = nc.dram_tensor("lg", [BL, P, N], F32, kind="ExternalInput")
    sc = nc.dram_tensor("sc", [BL, P, 20], F32, kind="ExternalInput")
    rw = nc.dram_tensor("rw", [1, 512], F32, kind="ExternalInput")
    sout = nc.dram_tensor("sout", [BL, P, N], F32, kind="ExternalOutput")
    cand = nc.dram_tensor("cand", [P, 2 * BL], F32, kind="ExternalOutput")

    with TileContext(nc) as tc:
        with (
            tc.tile_pool(name="big", bufs=1) as big,
            tc.tile_pool(name="sm", bufs=1) as sm,
            tc.tile_pool(name="ps", bufs=1, space="PSUM") as ps,
        ):
            # ---------------- small/const tiles ----------------
            sct = [sm.tile([P, 20], F32, tag=f"sct{b}", name=f"sct{b}")
                   for b in range(BL)]
            for b in range(BL):
                nc.sync.dma_start(out=sct[b][:], in_=sc[b])
            rwt = sm.tile([1, 512], F32, tag="rwt")
            nc.sync.dma_start(out=rwt[:], in_=rw[:])
            ident = sm.tile([120, 120], F32, tag="ident")
            make_identity(nc, ident[:])
            ones11 = sm.tile([1, 1], F32, tag="ones11")
            nc.vector.memset(ones11[:], 1.0)

            def col(b, i):  # per-partition scalar column for batch b
                return sct[b][:, i : i + 1]

            # ---------------- big input tiles ----------------
            ain = big.tile([P, NPLANES, N], F32, tag="ain")

            def v(j):
                return ain[:, j, :]

            ALT = [v(d) for d in range(3)]
            ARB = [v(3 + d) for d in range(3)]
            RS = [v(6 + d) for d in range(3)]
            VOLA = v(9)
            A = [v(10 + d) for d in range(6)]

            def load_group(j0, j1):
                nc.sync.dma_start(out=ain[:, j0:j1, :],
                                  in_=ath[j0:j1].rearrange("i p n -> p i n"))

            lgt = [big.tile([P, N], F32, tag=f"lg{b}", name=f"lg{b}")
                   for b in range(BL)]

            load_group(0, 6)      # alt, arb
            for b in range(BL):
                nc.sync.dma_start(out=lgt[b][:], in_=lg[b])
            load_group(6, 10)     # rs, vola
            load_group(10, 16)    # a0-5

            # per-batch working tiles (8 slots per batch, heavily reused)
            SMX = [big.tile([P, N], F32, tag=f"smx{b}", name=f"smx{b}")
                   for b in range(BL)]
            M = [[big.tile([P, N], F32, tag=f"m{b}_{i}", name=f"m{b}_{i}")
                  for i in range(3)] for b in range(BL)]
            VC = [[big.tile([P, N], F32, tag=f"vc{b}_{i}", name=f"vc{b}_{i}")
                   for i in range(3)] for b in range(BL)]
            UN = [big.tile([P, N], F32, tag=f"un{b}", name=f"un{b}")
                  for b in range(BL)]

            # ---------------- big passes (interleaved per batch) -----------
            # mx_d/m_d: S_mx is a rolling scratch (DVE-serial anyway)
            for d in range(3):
                for b in range(BL):
                    nc.vector.tensor_scalar_max(out=SMX[b][:], in0=ALT[d],
                                                scalar1=col(b, 6 + d))
                    nc.vector.scalar_tensor_tensor(
                        out=M[b][d][:], in0=ARB[d], scalar=col(b, 9 + d),
                        in1=SMX[b][:], op0=ALU.min, op1=ALU.subtract)
            for b in range(BL):
                nc.scalar.activation(lgt[b][:], lgt[b][:], ACTF.Sigmoid)
            for d in range(3):
                for b in range(BL):
                    nc.vector.scalar_tensor_tensor(
                        out=VC[b][d][:], in0=RS[d], scalar=col(b, 12 + d),
                        in1=M[b][d][:], op0=ALU.add, op1=ALU.subtract)
            for d in range(3):
                for b in range(BL):
                    nc.scalar.activation(M[b][d][:], M[b][d][:], ACTF.Relu)
            for b in range(BL):  # inter -> M0
                nc.gpsimd.tensor_tensor(out=M[b][0][:], in0=M[b][0][:],
                                        in1=M[b][1][:], op=ALU.mult)
                nc.gpsimd.tensor_tensor(out=M[b][0][:], in0=M[b][0][:],
                                        in1=M[b][2][:], op=ALU.mult)
            inter = [M[b][0] for b in range(BL)]
            for b in range(BL):  # union -> UN
                nc.vector.scalar_tensor_tensor(
                    out=UN[b][:], in0=VOLA, scalar=col(b, 15), in1=inter[b][:],
                    op0=ALU.add, op1=ALU.subtract)
            for b in range(BL):  # volc -> SMX
                nc.gpsimd.tensor_tensor(out=SMX[b][:], in0=VC[b][0][:],
                                        in1=VC[b][1][:], op=ALU.mult)
                nc.gpsimd.tensor_tensor(out=SMX[b][:], in0=SMX[b][:],
                                        in1=VC[b][2][:], op=ALU.mult)
            volc = SMX
            for b in range(BL):  # den -> M1
                nc.gpsimd.tensor_tensor(out=M[b][1][:], in0=UN[b][:],
                                        in1=volc[b][:], op=ALU.mult)
            for b in range(BL):  # rden -> M2
                nc.vector.reciprocal_approx_fast(out=M[b][2][:], in_=M[b][1][:])
            for b in range(BL):  # ivc in place over inter (M0)
                nc.gpsimd.tensor_tensor(out=inter[b][:], in0=inter[b][:],
                                        in1=volc[b][:], op=ALU.mult)
            for b in range(BL):  # u2 = union^2 in place over UN (after den)
                nc.gpsimd.tensor_tensor(out=UN[b][:], in0=UN[b][:],
                                        in1=UN[b][:], op=ALU.mult)
            for b in range(BL):  # num = u2 + ivc -> UN
                nc.vector.tensor_tensor(out=UN[b][:], in0=UN[b][:],
                                        in1=inter[b][:], op=ALU.add)
            for b in range(BL):  # frac = num * rden -> UN
                nc.vector.tensor_tensor(out=UN[b][:], in0=UN[b][:],
                                        in1=M[b][2][:], op=ALU.mult)
            frac = UN

            # cost_bbox: ab planes into VC0-2, M1, M2, SMX (ACT)
            abt = [[VC[b][0], VC[b][1], VC[b][2], M[b][1], M[b][2], SMX[b]]
                   for b in range(BL)]
            for d in range(6):
                for b in range(BL):
                    nc.scalar.activation(abt[b][d][:], A[d], ACTF.Abs,
                                         bias=col(b, d), scale=1.0)
            for b in range(BL):  # pair adds on GP
                nc.gpsimd.tensor_tensor(out=abt[b][0][:], in0=abt[b][0][:],
                                        in1=abt[b][1][:], op=ALU.add)
                nc.gpsimd.tensor_tensor(out=abt[b][2][:], in0=abt[b][2][:],
                                        in1=abt[b][3][:], op=ALU.add)
                nc.gpsimd.tensor_tensor(out=abt[b][4][:], in0=abt[b][4][:],
                                        in1=abt[b][5][:], op=ALU.add)
            for b in range(BL):  # joins on DVE
                nc.vector.tensor_tensor(out=abt[b][0][:], in0=abt[b][0][:],
                                        in1=abt[b][2][:], op=ALU.add)
                nc.vector.tensor_tensor(out=abt[b][0][:], in0=abt[b][0][:],
                                        in1=abt[b][4][:], op=ALU.add)
            cb = [abt[b][0] for b in range(BL)]
            # u1 = cb*-2.5 + sig; negc = u1 + frac (in place over lgt)
            for b in range(BL):
                nc.vector.scalar_tensor_tensor(
                    out=lgt[b][:], in0=cb[b][:], scalar=-2.5, in1=lgt[b][:],
                    op0=ALU.mult, op1=ALU.add)
                nc.vector.tensor_tensor(out=lgt[b][:], in0=lgt[b][:],
                                        in1=frac[b][:], op=ALU.add)
            negc = lgt

            # per-partition top-8 + index, frac min/max
            candt = sm.tile([P, 2 * BL], F32, tag="candt")
            mx8 = [sm.tile([P, 8], F32, tag=f"mx8_{b}", name=f"mx8_{b}")
                   for b in range(BL)]
            ix8 = [sm.tile([P, 8], mybir.dt.uint32, tag=f"ix8_{b}", name=f"ix8_{b}")
                   for b in range(BL)]
            fmx = [sm.tile([P, 1], F32, tag=f"fmx{b}", name=f"fmx{b}")
                   for b in range(BL)]
            fmn = [sm.tile([P, 1], F32, tag=f"fmn{b}", name=f"fmn{b}")
                   for b in range(BL)]
            for b in range(BL):
                nc.vector.max(out=mx8[b][:], in_=negc[b][:])
                nc.vector.max_index(out=ix8[b][:], in_max=mx8[b][:],
                                    in_values=negc[b][:])
                nc.vector.tensor_copy(out=candt[:, 2 * b : 2 * b + 1],
                                      in_=mx8[b][:, 0:1])
                ixf = sm.tile([P, 1], F32, tag=f"ixf{b}", name=f"ixf{b}")
                nc.vector.tensor_copy(out=ixf[:], in_=ix8[b][:, 0:1])
                nc.vector.tensor_scalar_add(out=candt[:, 2 * b + 1 : 2 * b + 2],
                                            in0=ixf[:], scalar1=col(b, 16))
                nc.vector.tensor_reduce(out=fmx[b][:], in_=frac[b][:],
                                        axis=AXL.X, op=ALU.max)
                nc.vector.tensor_reduce(out=fmn[b][:], in_=frac[b][:],
                                        axis=AXL.X, op=ALU.min)
            nc.sync.dma_start(out=cand[:], in_=candt[:])

            # ---------------- soft-label scale/bias (tiny, per batch) -------
            def g6(ap120):  # [1,120] -> [1,20,6]
                return ap120.rearrange("p (g c) -> p g c", c=NCH)

            def b6(ap20):  # [1,20] -> [1,20,6] broadcast read
                return ap20[:, :, None].broadcast_to((1, O, NCH))

            for b in range(BL):
                def srow(tag, w=120):
                    t = f"{tag}{b}"
                    return sm.tile([1, w], F32, tag=t, name=t)

                fx_t = ps.tile([1, 120], F32, tag=f"fx_t{b}", name=f"fx_t{b}")
                nc.tensor.transpose(fx_t[:], fmx[b][:], ident[:])
                fn_t = ps.tile([1, 120], F32, tag=f"fn_t{b}", name=f"fn_t{b}")
                nc.tensor.transpose(fn_t[:], fmn[b][:], ident[:])
                gfx = srow("gfx", O)
                gfn = srow("gfn", O)
                nc.vector.tensor_reduce(out=gfx[:], in_=g6(fx_t[:]),
                                        axis=AXL.X, op=ALU.max)
                nc.vector.tensor_reduce(out=gfn[:], in_=g6(fn_t[:]),
                                        axis=AXL.X, op=ALU.min)
                dd = srow("dd", O)
                nc.vector.tensor_tensor(out=dd[:], in0=gfx[:], in1=gfn[:],
                                        op=ALU.subtract)
                inv = srow("inv", O)
                nc.vector.reciprocal(out=inv[:], in_=dd[:])
                nb = srow("nb", O)
                nc.vector.tensor_tensor(out=nb[:], in0=gfn[:], in1=inv[:],
                                        op=ALU.mult)
                nc.vector.tensor_scalar_mul(out=nb[:], in0=nb[:], scalar1=-1.0)
                prs_row = rwt[:, 256 * b : 256 * b + 120]
                prsm1_row = rwt[:, 256 * b + 128 : 256 * b + 248]
                scale_r = srow("scale_r")
                nc.vector.tensor_tensor(out=g6(scale_r[:]), in0=b6(inv[:]),
                                        in1=g6(prs_row), op=ALU.mult)
                bias_r = srow("bias_r")
                nc.vector.tensor_tensor(out=g6(bias_r[:]), in0=b6(nb[:]),
                                        in1=g6(prs_row), op=ALU.mult)
                nc.vector.tensor_tensor(out=bias_r[:], in0=bias_r[:],
                                        in1=prsm1_row, op=ALU.add)
                csc = ps.tile([120, 1], F32, tag=f"csc{b}", name=f"csc{b}")
                nc.tensor.transpose(csc[:], scale_r[:], ones11[:])
                cbi = ps.tile([120, 1], F32, tag=f"cbi{b}", name=f"cbi{b}")
                nc.tensor.transpose(cbi[:], bias_r[:], ones11[:])
                scale_c = sm.tile([P, 1], F32, tag=f"scale_c{b}", name=f"scale_c{b}")
                nc.vector.tensor_copy(out=scale_c[:], in_=csc[:])
                bias_c = sm.tile([P, 1], F32, tag=f"bias_c{b}", name=f"bias_c{b}")
                nc.vector.tensor_copy(out=bias_c[:], in_=cbi[:])

                # slp = frac*scale + bias ; sl = max(slp, floor)  (DVE, fused)
                slt = VC[b][1]
                nc.vector.tensor_scalar(out=slt[:], in0=frac[b][:],
                                        scalar1=scale_c[:], scalar2=bias_c[:],
                                        op0=ALU.mult, op1=ALU.add)
                nc.vector.tensor_scalar_max(out=slt[:], in0=slt[:],
                                            scalar1=col(b, 17))
                nc.sync.dma_start(out=sout[b], in_=slt[:])

    nc.finalize()
    return nc


def _prep_host(pred_logits, anchors, target_boxes, target_present):
    f32 = np.float32
    A = np.ascontiguousarray(anchors.reshape(O, QP, 6).astype(f32, copy=False))
    pad = lambda x: np.pad(x, ((0, 0), (0, NCH * N - QP)), mode="edge")

    comp = [pad(A[:, :, d]) for d in range(6)]  # [20, 8196] each
    rc = [np.maximum(comp[d], f32(0)) for d in range(3)]
    rsz = [np.maximum(comp[3 + d], f32(0)) for d in range(3)]
    alt = [rc[d] - f32(0.5) * rsz[d] for d in range(3)]
    arb = [rc[d] + f32(0.5) * rsz[d] for d in range(3)]
    vola = (rsz[0] * rsz[1]) * rsz[2]
    planes = alt + arb + rsz + [vola] + comp
    ath = np.stack([p.reshape(P, N) for p in planes])
    ath = np.ascontiguousarray(ath, dtype=f32)

    lgs = pred_logits.reshape(BS, O, QP).astype(f32, copy=False)
    lgs = np.pad(lgs, ((0, 0), (0, 0), (0, NCH * N - QP)), mode="edge")
    lg_all = lgs.reshape(BS, P, N)

    t = target_boxes.astype(f32, copy=False)          # [BS, O, 6]
    tc_, ts_ = t[..., :3], t[..., 3:]
    blt = tc_ - f32(0.5) * ts_
    brb = tc_ + f32(0.5) * ts_
    fd = brb - blt
    volb = (fd[..., 0] * fd[..., 1]) * fd[..., 2]
    prs = target_present.astype(f32, copy=False)      # [BS, O]

    in_maps = []
    for c in range(NCORES):
        b0 = c * BL
        lgc = np.ascontiguousarray(lg_all[b0 : b0 + BL], dtype=f32)
        scv = np.zeros((BL, P, 20), f32)
        sc3 = scv.reshape(BL, O, NCH, 20)
        for b in range(BL):
            gb = b0 + b
            sc3[b, :, :, 0:6] = -t[gb][:, None, :]
            sc3[b, :, :, 6:9] = blt[gb][:, None, :]
            sc3[b, :, :, 9:12] = brb[gb][:, None, :]
            sc3[b, :, :, 12:15] = fd[gb][:, None, :]
            sc3[b, :, :, 15] = volb[gb][:, None]
            sc3[b, :, :, 16] = np.arange(NCH, dtype=f32)[None, :] * f32(N)
            sc3[b, :, :, 17] = prs[gb][:, None] - f32(1)  # floor
        rwv = np.zeros((1, 512), f32)
        for b in range(BL):
            pr6 = np.repeat(prs[b0 + b], NCH)
            rwv[0, 256 * b : 256 * b + 120] = pr6
            rwv[0, 256 * b + 128 : 256 * b + 248] = pr6 - f32(1)
        in_maps.append({"ath": ath, "lg": lgc, "sc": scv, "rw": rwv})
    return in_maps


def kernel(pred_logits, pred_boxes, anchors, target_boxes, target_present,
           num_top_queries):
    k = int(num_top_queries)
    assert k == 1, f"kernel specialized for num_top_queries=1, got {k}"

    if "nc" not in _BUILT:
        _BUILT["nc"] = _build_nc()
    nc = _BUILT["nc"]

    pred_logits = np.asarray(pred_logits)
    anchors = np.asarray(anchors)
    target_boxes = np.asarray(target_boxes)
    target_present = np.asarray(target_present)
    in_maps = _prep_host(pred_logits, anchors, target_boxes, target_present)
    res = run_bass_kernel_spmd(nc, in_maps, core_ids=list(range(NCORES)))

    matches = np.zeros((BS, O, QP), np.int32)
    soft = np.empty((BS, O, QP), np.float32)
    present = target_present.astype(bool)
    for c, r in enumerate(res.results):
        b0 = c * BL
        soft[b0 : b0 + BL] = r["sout"].reshape(BL, O, NCH * N)[:, :, :QP]
        cd = r["cand"].reshape(O, NCH, 2 * BL)
        for b in range(BL):
            vals = cd[:, :, 2 * b]          # [O, NCH] chunk-max of negC/2
            gidx = cd[:, :, 2 * b + 1]      # [O, NCH] global q of chunk winner
            win = np.argmax(vals, axis=1)   # first max -> lowest chunk on ties
            for o in range(O):
                if present[b0 + b, o]:
                    matches[b0 + b, o, int(gidx[o, win[o]])] = 1
    return matches, soft


# revision 10
# speedup vs baseline: 1.4091x; 1.1627x over previous
"""Trainium2 Bass kernel for the anchor-based NMS matcher.

Math (see problem reference): per (batch b, organ o), over Qp=8192 anchor
queries q:
    cost_class = -sigmoid(logit)
    cost_bbox  = sum_d |anchor_d - tgt_d|            (cxcyczwhd space)
    cost_giou  = -giou3d(xyzxyz(clip(anchor,0)), xyzxyz(tgt))
    C = 5*cb + 2*cc + 2*cg
    matches     = one_hot(argmin_q C) * present
    soft_labels = present ? clip((cg-cgmax)/(cgmin-cgmax), 0) : -1

Device strategy (8 cores, data-parallel over batch, 2 batch items/core):
  SBUF layout: 120 partitions = (organ 20) x (q-chunk 6), free dim
  N=1366 (6*1366=8196, q padded 8192->8196 with edge dup).  The two batch
  items run as two interleaved half-width pass sets over the SAME anchor
  planes (loaded once -- no batch duplication of the big inputs).
  All per-(b,o) target quantities are per-partition scalars, enabling fused
  tensor_scalar / scalar_tensor_tensor / activation(bias,scale) ops.
  Anchor-derived planes (clipped lt/rb/size/vol) are precomputed on host.
  giou needs one reciprocal via
      -giou + 1 = 1 - (u^2 + inter*vol_c)/(u*vol_c),  u = union
  ranking with negC = sig - 2.5*cb + frac (argmax negC == argmin C); soft
  labels are normalized in frac-space (affine-invariant).
  Per-partition argmax via DVE max/max_index.  Per-chunk winner (value,
  global q) columns are DMA'd out and the 6-chunk combine + one-hot scatter
  happen on host (40 rows/core).  The soft-label scale/bias columns are
  produced on device via tiny PE transposes ([120,1] <-> [1,120]) so the
  cross-chunk stats logic runs on partition-0 row vectors.
"""

import numpy as np

import concourse.bacc as bacc
import concourse.bass as bass
import concourse.mybir as mybir
from concourse.bass_utils import run_bass_kernel_spmd
from concourse.masks import make_identity
from concourse.tile import TileContext

F32 = mybir.dt.float32
ALU = mybir.AluOpType
ACTF = mybir.ActivationFunctionType
AXL = mybir.AxisListType

BS, O, QP = 16, 20, 8192
NCORES = 8
BL = BS // NCORES        # batch items per core
NCH = 6                  # q chunks per organ
N = 1366                 # chunk width; 6*1366 = 8196 = 8192 + 4 pad
P = O * NCH              # 120 partitions
NPLANES = 16             # alt0-2, arb0-2, rs0-2, vola, a0-5

_BUILT = {}


def _build_nc():
    nc = bacc.Bacc("TRN2", target_bir_lowering=False, debug=False)
    ath = nc.dram_tensor("ath", [NPLANES, P, N], F32, kind="ExternalInput")
    lg = nc.dram_tensor("lg", [BL, P, N], F32, kind="ExternalInput")
    sc = nc.dram_tensor("sc", [BL, P, 20], F32, kind="ExternalInput")
    rw = nc.dram_tensor("rw", [1, 512], F32, kind="ExternalInput")
    sout = nc.dram_tensor("sout", [BL, P, N], F32, kind="ExternalOutput")
    cand = nc.dram_tensor("cand", [P, 2 * BL], F32, kind="ExternalOutput")

    with TileContext(nc) as tc:
        with (
            tc.tile_pool(name="big", bufs=1) as big,
            tc.tile_pool(name="sm", bufs=1) as sm,
            tc.tile_pool(name="ps", bufs=1, space="PSUM") as ps,
        ):
            # ---------------- small/const tiles ----------------
            sct = [sm.tile([P, 20], F32, tag=f"sct{b}", name=f"sct{b}")
                   for b in range(BL)]
            for b in range(BL):
                nc.sync.dma_start(out=sct[b][:], in_=sc[b])
            rwt = sm.tile([1, 512], F32, tag="rwt")
            nc.sync.dma_start(out=rwt[:], in_=rw[:])
            ident = sm.tile([120, 120], F32, tag="ident")
            make_identity(nc, ident[:])
            ones11 = sm.tile([1, 1], F32, tag="ones11")
            nc.vector.memset(ones11[:], 1.0)

            def col(b, i):  # per-partition scalar column for batch b
                return sct[b][:, i : i + 1]

            # ---------------- big input tiles ----------------
            ain = big.tile([P, NPLANES, N], F32, tag="ain")

            def v(j):
                return ain[:, j, :]

            ALT = [v(d) for d in range(3)]
            ARB = [v(3 + d) for d in range(3)]
            RS = [v(6 + d) for d in range(3)]
            VOLA = v(9)
            A = [v(10 + d) for d in range(6)]

            def load_group(j0, j1):
                nc.sync.dma_start(out=ain[:, j0:j1, :],
                                  in_=ath[j0:j1].rearrange("i p n -> p i n"))

            lgt = [big.tile([P, N], F32, tag=f"lg{b}", name=f"lg{b}")
                   for b in range(BL)]

            load_group(0, 3)      # alt
            load_group(3, 6)      # arb
            load_group(6, 10)     # rs, vola
            load_group(10, 16)    # a0-5
            for b in range(BL):
                nc.sync.dma_start(out=lgt[b][:], in_=lg[b])

            # per-batch working tiles (8 slots per batch, heavily reused)
            SMX = [big.tile([P, N], F32, tag=f"smx{b}", name=f"smx{b}")
                   for b in range(BL)]
            M = [[big.tile([P, N], F32, tag=f"m{b}_{i}", name=f"m{b}_{i}")
                  for i in range(3)] for b in range(BL)]
            VC = [[big.tile([P, N], F32, tag=f"vc{b}_{i}", name=f"vc{b}_{i}")
                   for i in range(3)] for b in range(BL)]
            UN = [big.tile([P, N], F32, tag=f"un{b}", name=f"un{b}")
                  for b in range(BL)]

            # ---------------- big passes (interleaved per batch) -----------
            # mx_d/m_d: S_mx is a rolling scratch (DVE-serial anyway)
            for d in range(3):
                for b in range(BL):
                    nc.vector.tensor_scalar_max(out=SMX[b][:], in0=ALT[d],
                                                scalar1=col(b, 6 + d))
                    nc.vector.scalar_tensor_tensor(
                        out=M[b][d][:], in0=ARB[d], scalar=col(b, 9 + d),
                        in1=SMX[b][:], op0=ALU.min, op1=ALU.subtract)
            for b in range(BL):
                nc.scalar.activation(lgt[b][:], lgt[b][:], ACTF.Sigmoid)
            for d in range(3):
                for b in range(BL):
                    nc.vector.scalar_tensor_tensor(
                        out=VC[b][d][:], in0=RS[d], scalar=col(b, 12 + d),
                        in1=M[b][d][:], op0=ALU.add, op1=ALU.subtract)
            for d in range(3):
                for b in range(BL):
                    nc.scalar.activation(M[b][d][:], M[b][d][:], ACTF.Relu)
            for b in range(BL):  # inter -> M0
                nc.gpsimd.tensor_tensor(out=M[b][0][:], in0=M[b][0][:],
                                        in1=M[b][1][:], op=ALU.mult)
                nc.gpsimd.tensor_tensor(out=M[b][0][:], in0=M[b][0][:],
                                        in1=M[b][2][:], op=ALU.mult)
            inter = [M[b][0] for b in range(BL)]
            for b in range(BL):  # union -> UN
                nc.vector.scalar_tensor_tensor(
                    out=UN[b][:], in0=VOLA, scalar=col(b, 15), in1=inter[b][:],
                    op0=ALU.add, op1=ALU.subtract)
            for b in range(BL):  # volc -> SMX
                nc.gpsimd.tensor_tensor(out=SMX[b][:], in0=VC[b][0][:],
                                        in1=VC[b][1][:], op=ALU.mult)
                nc.gpsimd.tensor_tensor(out=SMX[b][:], in0=SMX[b][:],
                                        in1=VC[b][2][:], op=ALU.mult)
            volc = SMX
            for b in range(BL):  # den -> M1
                nc.gpsimd.tensor_tensor(out=M[b][1][:], in0=UN[b][:],
                                        in1=volc[b][:], op=ALU.mult)
            for b in range(BL):  # rden -> M2
                nc.vector.reciprocal_approx_fast(out=M[b][2][:], in_=M[b][1][:])
            for b in range(BL):  # ivc in place over inter (M0)
                nc.gpsimd.tensor_tensor(out=inter[b][:], in0=inter[b][:],
                                        in1=volc[b][:], op=ALU.mult)
            for b in range(BL):  # u2 = union^2 in place over UN (after den)
                nc.scalar.activation(UN[b][:], UN[b][:], ACTF.Square)
            for b in range(BL):  # num = u2 + ivc -> UN
                nc.vector.tensor_tensor(out=UN[b][:], in0=UN[b][:],
                                        in1=inter[b][:], op=ALU.add)
            for b in range(BL):  # frac = num * rden -> UN
                nc.vector.tensor_tensor(out=UN[b][:], in0=UN[b][:],
                                        in1=M[b][2][:], op=ALU.mult)
            frac = UN

            # ---------------- soft-label path first (gates last output) ---
            fmx = [sm.tile([P, 1], F32, tag=f"fmx{b}", name=f"fmx{b}")
                   for b in range(BL)]
            fmn = [sm.tile([P, 1], F32, tag=f"fmn{b}", name=f"fmn{b}")
                   for b in range(BL)]
            for b in range(BL):
                nc.vector.tensor_reduce(out=fmx[b][:], in_=frac[b][:],
                                        axis=AXL.X, op=ALU.max)
                nc.vector.tensor_reduce(out=fmn[b][:], in_=frac[b][:],
                                        axis=AXL.X, op=ALU.min)

            def g6(ap120):  # [1,120] -> [1,20,6]
                return ap120.rearrange("p (g c) -> p g c", c=NCH)

            def b6(ap20):  # [1,20] -> [1,20,6] broadcast read
                return ap20[:, :, None].broadcast_to((1, O, NCH))

            for b in range(BL):
                def srow(tag, w=120):
                    t = f"{tag}{b}"
                    return sm.tile([1, w], F32, tag=t, name=t)

                fx_t = ps.tile([1, 120], F32, tag=f"fx_t{b}", name=f"fx_t{b}")
                nc.tensor.transpose(fx_t[:], fmx[b][:], ident[:])
                fn_t = ps.tile([1, 120], F32, tag=f"fn_t{b}", name=f"fn_t{b}")
                nc.tensor.transpose(fn_t[:], fmn[b][:], ident[:])
                gfx = srow("gfx", O)
                gfn = srow("gfn", O)
                nc.vector.tensor_reduce(out=gfx[:], in_=g6(fx_t[:]),
                                        axis=AXL.X, op=ALU.max)
                nc.vector.tensor_reduce(out=gfn[:], in_=g6(fn_t[:]),
                                        axis=AXL.X, op=ALU.min)
                dd = srow("dd", O)
                nc.vector.tensor_tensor(out=dd[:], in0=gfx[:], in1=gfn[:],
                                        op=ALU.subtract)
                inv = srow("inv", O)
                nc.vector.reciprocal(out=inv[:], in_=dd[:])
                nb = srow("nb", O)
                nc.vector.tensor_tensor(out=nb[:], in0=gfn[:], in1=inv[:],
                                        op=ALU.mult)
                nc.vector.tensor_scalar_mul(out=nb[:], in0=nb[:], scalar1=-1.0)
                prs_row = rwt[:, 256 * b : 256 * b + 120]
                prsm1_row = rwt[:, 256 * b + 128 : 256 * b + 248]
                scale_r = srow("scale_r")
                nc.vector.tensor_tensor(out=g6(scale_r[:]), in0=b6(inv[:]),
                                        in1=g6(prs_row), op=ALU.mult)
                bias_r = srow("bias_r")
                nc.vector.tensor_tensor(out=g6(bias_r[:]), in0=b6(nb[:]),
                                        in1=g6(prs_row), op=ALU.mult)
                nc.vector.tensor_tensor(out=bias_r[:], in0=bias_r[:],
                                        in1=prsm1_row, op=ALU.add)
                csc = ps.tile([120, 1], F32, tag=f"csc{b}", name=f"csc{b}")
                nc.tensor.transpose(csc[:], scale_r[:], ones11[:])
                cbi = ps.tile([120, 1], F32, tag=f"cbi{b}", name=f"cbi{b}")
                nc.tensor.transpose(cbi[:], bias_r[:], ones11[:])
                scale_c = sm.tile([P, 1], F32, tag=f"scale_c{b}", name=f"scale_c{b}")
                nc.vector.tensor_copy(out=scale_c[:], in_=csc[:])
                bias_c = sm.tile([P, 1], F32, tag=f"bias_c{b}", name=f"bias_c{b}")
                nc.vector.tensor_copy(out=bias_c[:], in_=cbi[:])

                # slp = frac*scale + bias ; sl = max(slp, floor) -> M0 (free)
                slt = M[b][0]
                nc.vector.tensor_scalar(out=slt[:], in0=frac[b][:],
                                        scalar1=scale_c[:], scalar2=bias_c[:],
                                        op0=ALU.mult, op1=ALU.add)
                nc.vector.tensor_scalar_max(out=slt[:], in0=slt[:],
                                            scalar1=col(b, 17))
                nc.sync.dma_start(out=sout[b], in_=slt[:])

            # ---------------- cost_bbox + ranking --------------------------
            # ab planes into VC0-2, M1, M2, SMX (ACT)
            abt = [[VC[b][0], VC[b][1], VC[b][2], M[b][1], M[b][2], SMX[b]]
                   for b in range(BL)]
            for d in range(6):
                for b in range(BL):
                    nc.scalar.activation(abt[b][d][:], A[d], ACTF.Abs,
                                         bias=col(b, d), scale=1.0)
            for b in range(BL):  # pair adds on GP
                nc.gpsimd.tensor_tensor(out=abt[b][0][:], in0=abt[b][0][:],
                                        in1=abt[b][1][:], op=ALU.add)
                nc.gpsimd.tensor_tensor(out=abt[b][2][:], in0=abt[b][2][:],
                                        in1=abt[b][3][:], op=ALU.add)
                nc.gpsimd.tensor_tensor(out=abt[b][4][:], in0=abt[b][4][:],
                                        in1=abt[b][5][:], op=ALU.add)
            for b in range(BL):  # joins on DVE
                nc.vector.tensor_tensor(out=abt[b][0][:], in0=abt[b][0][:],
                                        in1=abt[b][2][:], op=ALU.add)
                nc.vector.tensor_tensor(out=abt[b][0][:], in0=abt[b][0][:],
                                        in1=abt[b][4][:], op=ALU.add)
            cb = [abt[b][0] for b in range(BL)]
            # u1 = cb*-2.5 + sig; negc = u1 + frac (in place over lgt)
            for b in range(BL):
                nc.vector.scalar_tensor_tensor(
                    out=lgt[b][:], in0=cb[b][:], scalar=-2.5, in1=lgt[b][:],
                    op0=ALU.mult, op1=ALU.add)
                nc.vector.tensor_tensor(out=lgt[b][:], in0=lgt[b][:],
                                        in1=frac[b][:], op=ALU.add)
            negc = lgt

            # per-partition top-8 + index -> cand columns
            candt = sm.tile([P, 2 * BL], F32, tag="candt")
            for b in range(BL):
                mx8 = sm.tile([P, 8], F32, tag=f"mx8_{b}", name=f"mx8_{b}")
                ix8 = sm.tile([P, 8], mybir.dt.uint32, tag=f"ix8_{b}",
                              name=f"ix8_{b}")
                nc.vector.max(out=mx8[:], in_=negc[b][:])
                nc.vector.max_index(out=ix8[:], in_max=mx8[:],
                                    in_values=negc[b][:])
                nc.vector.tensor_copy(out=candt[:, 2 * b : 2 * b + 1],
                                      in_=mx8[:, 0:1])
                ixf = sm.tile([P, 1], F32, tag=f"ixf{b}", name=f"ixf{b}")
                nc.vector.tensor_copy(out=ixf[:], in_=ix8[:, 0:1])
                nc.vector.tensor_scalar_add(out=candt[:, 2 * b + 1 : 2 * b + 2],
                                            in0=ixf[:], scalar1=col(b, 16))
            nc.sync.dma_start(out=cand[:], in_=candt[:])

    nc.finalize()
    return nc


def _prep_host(pred_logits, anchors, target_boxes, target_present):
    f32 = np.float32
    A = np.ascontiguousarray(anchors.reshape(O, QP, 6).astype(f32, copy=False))
    pad = lambda x: np.pad(x, ((0, 0), (0, NCH * N - QP)), mode="edge")

    comp = [pad(A[:, :, d]) for d in range(6)]  # [20, 8196] each
    rc = [np.maximum(comp[d], f32(0)) for d in range(3)]
    rsz = [np.maximum(comp[3 + d], f32(0)) for d in range(3)]
    alt = [rc[d] - f32(0.5) * rsz[d] for d in range(3)]
    arb = [rc[d] + f32(0.5) * rsz[d] for d in range(3)]
    vola = (rsz[0] * rsz[1]) * rsz[2]
    planes = alt + arb + rsz + [vola] + comp
    ath = np.stack([p.reshape(P, N) for p in planes])
    ath = np.ascontiguousarray(ath, dtype=f32)

    lgs = pred_logits.reshape(BS, O, QP).astype(f32, copy=False)
    lgs = np.pad(lgs, ((0, 0), (0, 0), (0, NCH * N - QP)), mode="edge")
    lg_all = lgs.reshape(BS, P, N)

    t = target_boxes.astype(f32, copy=False)          # [BS, O, 6]
    tc_, ts_ = t[..., :3], t[..., 3:]
    blt = tc_ - f32(0.5) * ts_
    brb = tc_ + f32(0.5) * ts_
    fd = brb - blt
    volb = (fd[..., 0] * fd[..., 1]) * fd[..., 2]
    prs = target_present.astype(f32, copy=False)      # [BS, O]

    in_maps = []
    for c in range(NCORES):
        b0 = c * BL
        lgc = np.ascontiguousarray(lg_all[b0 : b0 + BL], dtype=f32)
        scv = np.zeros((BL, P, 20), f32)
        sc3 = scv.reshape(BL, O, NCH, 20)
        for b in range(BL):
            gb = b0 + b
            sc3[b, :, :, 0:6] = -t[gb][:, None, :]
            sc3[b, :, :, 6:9] = blt[gb][:, None, :]
            sc3[b, :, :, 9:12] = brb[gb][:, None, :]
            sc3[b, :, :, 12:15] = fd[gb][:, None, :]
            sc3[b, :, :, 15] = volb[gb][:, None]
            sc3[b, :, :, 16] = np.arange(NCH, dtype=f32)[None, :] * f32(N)
            sc3[b, :, :, 17] = prs[gb][:, None] - f32(1)  # floor
        rwv = np.zeros((1, 512), f32)
        for b in range(BL):
            pr6 = np.repeat(prs[b0 + b], NCH)
            rwv[0, 256 * b : 256 * b + 120] = pr6
            rwv[0, 256 * b + 128 : 256 * b + 248] = pr6 - f32(1)
        in_maps.append({"ath": ath, "lg": lgc, "sc": scv, "rw": rwv})
    return in_maps


def kernel(pred_logits, pred_boxes, anchors, target_boxes, target_present,
           num_top_queries):
    k = int(num_top_queries)
    assert k == 1, f"kernel specialized for num_top_queries=1, got {k}"

    if "nc" not in _BUILT:
        _BUILT["nc"] = _build_nc()
    nc = _BUILT["nc"]

    pred_logits = np.asarray(pred_logits)
    anchors = np.asarray(anchors)
    target_boxes = np.asarray(target_boxes)
    target_present = np.asarray(target_present)
    in_maps = _prep_host(pred_logits, anchors, target_boxes, target_present)
    res = run_bass_kernel_spmd(nc, in_maps, core_ids=list(range(NCORES)))

    matches = np.zeros((BS, O, QP), np.int32)
    soft = np.empty((BS, O, QP), np.float32)
    present = target_present.astype(bool)
    for c, r in enumerate(res.results):
        b0 = c * BL
        soft[b0 : b0 + BL] = r["sout"].reshape(BL, O, NCH * N)[:, :, :QP]
        cd = r["cand"].reshape(O, NCH, 2 * BL)
        for b in range(BL):
            vals = cd[:, :, 2 * b]          # [O, NCH] chunk-max of negC/2
            gidx = cd[:, :, 2 * b + 1]      # [O, NCH] global q of chunk winner
            win = np.argmax(vals, axis=1)   # first max -> lowest chunk on ties
            for o in range(O):
                if present[b0 + b, o]:
                    matches[b0 + b, o, int(gidx[o, win[o]])] = 1
    return matches, soft


# revision 12
# speedup vs baseline: 1.4468x; 1.0268x over previous
"""Trainium2 Bass kernel for the anchor-based NMS matcher.

Math (see problem reference): per (batch b, organ o), over Qp=8192 anchor
queries q:
    cost_class = -sigmoid(logit)
    cost_bbox  = sum_d |anchor_d - tgt_d|            (cxcyczwhd space)
    cost_giou  = -giou3d(xyzxyz(clip(anchor,0)), xyzxyz(tgt))
    C = 5*cb + 2*cc + 2*cg
    matches     = one_hot(argmin_q C) * present
    soft_labels = present ? clip((cg-cgmax)/(cgmin-cgmax), 0) : -1

Device strategy (8 cores, data-parallel over batch, 2 batch items/core):
  SBUF layout: 120 partitions = (organ 20) x (q-chunk 6), free dim
  N=1366 (6*1366=8196, q padded 8192->8196 with edge dup).  The two batch
  items run as two interleaved half-width pass sets over the SAME anchor
  planes (loaded once -- no batch duplication of the big inputs).
  All per-(b,o) target quantities are per-partition scalars, enabling fused
  tensor_scalar / scalar_tensor_tensor / activation(bias,scale) ops.
  Anchor-derived planes (clipped lt/rb/size/vol) are precomputed on host.
  giou needs one reciprocal via
      -giou + 1 = 1 - (u^2 + inter*vol_c)/(u*vol_c),  u = union
  ranking with negC = sig - 2.5*cb + frac (argmax negC == argmin C); soft
  labels are normalized in frac-space (affine-invariant).
  Per-partition argmax via DVE max/max_index.  Per-chunk winner (value,
  global q) columns are DMA'd out and the 6-chunk combine + one-hot scatter
  happen on host (40 rows/core).  The soft-label scale/bias columns are
  produced on device via tiny PE transposes ([120,1] <-> [1,120]) so the
  cross-chunk stats logic runs on partition-0 row vectors.
"""

import numpy as np

import concourse.bacc as bacc
import concourse.bass as bass
import concourse.mybir as mybir
from concourse.bass_utils import run_bass_kernel_spmd
from concourse.masks import make_identity
from concourse.tile import TileContext

F32 = mybir.dt.float32
ALU = mybir.AluOpType
ACTF = mybir.ActivationFunctionType
AXL = mybir.AxisListType

BS, O, QP = 16, 20, 8192
NCORES = 8
BL = BS // NCORES        # batch items per core
NCH = 6                  # q chunks per organ
N = 1366                 # chunk width; 6*1366 = 8196 = 8192 + 4 pad
P = O * NCH              # 120 partitions
NPLANES = 16             # alt0-2, arb0-2, rs0-2, vola, a0-5

_BUILT = {}


def _build_nc():
    nc = bacc.Bacc("TRN2", target_bir_lowering=False, debug=False)
    ath = nc.dram_tensor("ath", [NPLANES, P, N], F32, kind="ExternalInput")
    lg = nc.dram_tensor("lg", [BL, P, N], F32, kind="ExternalInput")
    sc = nc.dram_tensor("sc", [BL, P, 20], F32, kind="ExternalInput")
    rw = nc.dram_tensor("rw", [1, 512], F32, kind="ExternalInput")
    sout = nc.dram_tensor("sout", [BL, P, N], F32, kind="ExternalOutput")
    cand = nc.dram_tensor("cand", [P, 2 * BL], F32, kind="ExternalOutput")

    with TileContext(nc) as tc:
        with (
            tc.tile_pool(name="big", bufs=1) as big,
            tc.tile_pool(name="sm", bufs=1) as sm,
            tc.tile_pool(name="ps", bufs=1, space="PSUM") as ps,
        ):
            # ---------------- small/const tiles ----------------
            sct = [sm.tile([P, 20], F32, tag=f"sct{b}", name=f"sct{b}")
                   for b in range(BL)]
            for b in range(BL):
                nc.sync.dma_start(out=sct[b][:], in_=sc[b])
            rwt = sm.tile([1, 512], F32, tag="rwt")
            nc.sync.dma_start(out=rwt[:], in_=rw[:])
            ident = sm.tile([120, 120], F32, tag="ident")
            make_identity(nc, ident[:])
            ones11 = sm.tile([1, 1], F32, tag="ones11")
            nc.vector.memset(ones11[:], 1.0)

            def col(b, i):  # per-partition scalar column for batch b
                return sct[b][:, i : i + 1]

            # ---------------- big input tiles ----------------
            ain = big.tile([P, NPLANES, N], F32, tag="ain")

            def v(j):
                return ain[:, j, :]

            ALT = [v(d) for d in range(3)]
            ARB = [v(3 + d) for d in range(3)]
            RS = [v(6 + d) for d in range(3)]
            VOLA = v(9)
            A = [v(10 + d) for d in range(6)]

            def load_group(j0, j1):
                nc.sync.dma_start(out=ain[:, j0:j1, :],
                                  in_=ath[j0:j1].rearrange("i p n -> p i n"))

            lgt = [big.tile([P, N], F32, tag=f"lg{b}", name=f"lg{b}")
                   for b in range(BL)]

            load_group(0, 3)      # alt
            load_group(3, 6)      # arb
            load_group(6, 10)     # rs, vola
            load_group(10, 16)    # a0-5
            for b in range(BL):
                nc.sync.dma_start(out=lgt[b][:], in_=lg[b])

            # per-batch working tiles (8 slots per batch, heavily reused)
            SMX = [big.tile([P, N], F32, tag=f"smx{b}", name=f"smx{b}")
                   for b in range(BL)]
            M = [[big.tile([P, N], F32, tag=f"m{b}_{i}", name=f"m{b}_{i}")
                  for i in range(3)] for b in range(BL)]
            VC = [[big.tile([P, N], F32, tag=f"vc{b}_{i}", name=f"vc{b}_{i}")
                   for i in range(3)] for b in range(BL)]
            UN = [big.tile([P, N], F32, tag=f"un{b}", name=f"un{b}")
                  for b in range(BL)]

            # ---------------- big passes (interleaved per batch) -----------
            # mx_d/m_d: S_mx is a rolling scratch (DVE-serial anyway)
            for d in range(3):
                for b in range(BL):
                    nc.vector.tensor_scalar_max(out=SMX[b][:], in0=ALT[d],
                                                scalar1=col(b, 6 + d))
                    nc.vector.scalar_tensor_tensor(
                        out=M[b][d][:], in0=ARB[d], scalar=col(b, 9 + d),
                        in1=SMX[b][:], op0=ALU.min, op1=ALU.subtract)
            for b in range(BL):
                nc.scalar.activation(lgt[b][:], lgt[b][:], ACTF.Sigmoid)
            for d in range(3):
                for b in range(BL):
                    nc.vector.scalar_tensor_tensor(
                        out=VC[b][d][:], in0=RS[d], scalar=col(b, 12 + d),
                        in1=M[b][d][:], op0=ALU.add, op1=ALU.subtract)
            for d in range(3):
                for b in range(BL):
                    nc.scalar.activation(M[b][d][:], M[b][d][:], ACTF.Relu)
            for b in range(BL):  # inter -> M0
                nc.gpsimd.tensor_tensor(out=M[b][0][:], in0=M[b][0][:],
                                        in1=M[b][1][:], op=ALU.mult)
                nc.gpsimd.tensor_tensor(out=M[b][0][:], in0=M[b][0][:],
                                        in1=M[b][2][:], op=ALU.mult)
            inter = [M[b][0] for b in range(BL)]
            for b in range(BL):  # union -> UN
                nc.vector.scalar_tensor_tensor(
                    out=UN[b][:], in0=VOLA, scalar=col(b, 15), in1=inter[b][:],
                    op0=ALU.add, op1=ALU.subtract)
            for b in range(BL):  # volc -> SMX
                nc.gpsimd.tensor_tensor(out=SMX[b][:], in0=VC[b][0][:],
                                        in1=VC[b][1][:], op=ALU.mult)
                nc.gpsimd.tensor_tensor(out=SMX[b][:], in0=SMX[b][:],
                                        in1=VC[b][2][:], op=ALU.mult)
            volc = SMX
            for b in range(BL):  # den -> M1 (DVE: GP is the bottleneck here)
                nc.vector.tensor_tensor(out=M[b][1][:], in0=UN[b][:],
                                        in1=volc[b][:], op=ALU.mult)
            for b in range(BL):  # rden -> M2
                nc.vector.reciprocal_approx_fast(out=M[b][2][:], in_=M[b][1][:])
            for b in range(BL):  # ivc in place over inter (M0)
                nc.vector.tensor_tensor(out=inter[b][:], in0=inter[b][:],
                                        in1=volc[b][:], op=ALU.mult)
            for b in range(BL):  # u2 = union^2 in place over UN (after den)
                nc.scalar.activation(UN[b][:], UN[b][:], ACTF.Square)
            for b in range(BL):  # num = u2 + ivc -> UN
                nc.vector.tensor_tensor(out=UN[b][:], in0=UN[b][:],
                                        in1=inter[b][:], op=ALU.add)
            for b in range(BL):  # frac = num * rden -> UN
                nc.vector.tensor_tensor(out=UN[b][:], in0=UN[b][:],
                                        in1=M[b][2][:], op=ALU.mult)
            frac = UN

            # ---------------- soft-label path first (gates last output) ---
            fmx = [sm.tile([P, 1], F32, tag=f"fmx{b}", name=f"fmx{b}")
                   for b in range(BL)]
            fmn = [sm.tile([P, 1], F32, tag=f"fmn{b}", name=f"fmn{b}")
                   for b in range(BL)]
            for b in range(BL):
                nc.vector.tensor_reduce(out=fmx[b][:], in_=frac[b][:],
                                        axis=AXL.X, op=ALU.max)
                nc.vector.tensor_reduce(out=fmn[b][:], in_=frac[b][:],
                                        axis=AXL.X, op=ALU.min)

            def g6(ap120):  # [1,120] -> [1,20,6]
                return ap120.rearrange("p (g c) -> p g c", c=NCH)

            def b6(ap20):  # [1,20] -> [1,20,6] broadcast read
                return ap20[:, :, None].broadcast_to((1, O, NCH))

            for b in range(BL):
                def srow(tag, w=120):  # shared slots across batches (serial use)
                    return sm.tile([1, w], F32, tag=tag, name=tag)

                fx_t = ps.tile([1, 120], F32, tag=f"fx_t{b}", name=f"fx_t{b}")
                nc.tensor.transpose(fx_t[:], fmx[b][:], ident[:])
                fn_t = ps.tile([1, 120], F32, tag=f"fn_t{b}", name=f"fn_t{b}")
                nc.tensor.transpose(fn_t[:], fmn[b][:], ident[:])
                gfx = srow("gfx", O)
                gfn = srow("gfn", O)
                nc.vector.tensor_reduce(out=gfx[:], in_=g6(fx_t[:]),
                                        axis=AXL.X, op=ALU.max)
                nc.vector.tensor_reduce(out=gfn[:], in_=g6(fn_t[:]),
                                        axis=AXL.X, op=ALU.min)
                dd = srow("dd", O)
                nc.vector.tensor_tensor(out=dd[:], in0=gfx[:], in1=gfn[:],
                                        op=ALU.subtract)
                inv = srow("inv", O)
                nc.vector.reciprocal(out=inv[:], in_=dd[:])
                nb = srow("nb", O)
                nc.vector.tensor_tensor(out=nb[:], in0=gfn[:], in1=inv[:],
                                        op=ALU.mult)
                nc.vector.tensor_scalar_mul(out=nb[:], in0=nb[:], scalar1=-1.0)
                prs_row = rwt[:, 256 * b : 256 * b + 120]
                prsm1_row = rwt[:, 256 * b + 128 : 256 * b + 248]
                scale_r = srow("scale_r")
                nc.vector.tensor_tensor(out=g6(scale_r[:]), in0=b6(inv[:]),
                                        in1=g6(prs_row), op=ALU.mult)
                bias_r = srow("bias_r")
                nc.vector.tensor_tensor(out=g6(bias_r[:]), in0=b6(nb[:]),
                                        in1=g6(prs_row), op=ALU.mult)
                nc.vector.tensor_tensor(out=bias_r[:], in0=bias_r[:],
                                        in1=prsm1_row, op=ALU.add)
                csc = ps.tile([120, 1], F32, tag=f"csc{b}", name=f"csc{b}")
                nc.tensor.transpose(csc[:], scale_r[:], ones11[:])
                cbi = ps.tile([120, 1], F32, tag=f"cbi{b}", name=f"cbi{b}")
                nc.tensor.transpose(cbi[:], bias_r[:], ones11[:])
                scale_c = sm.tile([P, 1], F32, tag="scale_c", name="scale_c")
                nc.vector.tensor_copy(out=scale_c[:], in_=csc[:])
                bias_c = sm.tile([P, 1], F32, tag="bias_c", name="bias_c")
                nc.vector.tensor_copy(out=bias_c[:], in_=cbi[:])

                # slp = frac*scale + bias ; sl = max(slp, floor) -> M0 (free)
                slt = M[b][0]
                nc.vector.tensor_scalar(out=slt[:], in0=frac[b][:],
                                        scalar1=scale_c[:], scalar2=bias_c[:],
                                        op0=ALU.mult, op1=ALU.add)
                nc.vector.tensor_scalar_max(out=slt[:], in0=slt[:],
                                            scalar1=col(b, 17))
                nc.sync.dma_start(out=sout[b], in_=slt[:])

            # ---------------- cost_bbox + ranking --------------------------
            # ab0/ab1 get fresh slots (start as soon as a-planes land);
            # ab2-5 reuse VC0-2 + M1 after the giou chain frees them
            AB = [[big.tile([P, N], F32, tag=f"ab{b}_{i}", name=f"ab{b}_{i}")
                   for i in range(2)] for b in range(BL)]
            abt = [[AB[b][0], AB[b][1], VC[b][0], VC[b][1], VC[b][2], M[b][1]]
                   for b in range(BL)]
            for d in range(2):
                for b in range(BL):
                    nc.scalar.activation(abt[b][d][:], A[d], ACTF.Abs,
                                         bias=col(b, d), scale=1.0)
            for b in range(BL):  # t1 early on GP
                nc.gpsimd.tensor_tensor(out=abt[b][0][:], in0=abt[b][0][:],
                                        in1=abt[b][1][:], op=ALU.add)
            for d in range(2, 6):
                for b in range(BL):
                    nc.scalar.activation(abt[b][d][:], A[d], ACTF.Abs,
                                         bias=col(b, d), scale=1.0)
            for b in range(BL):  # t2/t3 on GP
                nc.gpsimd.tensor_tensor(out=abt[b][2][:], in0=abt[b][2][:],
                                        in1=abt[b][3][:], op=ALU.add)
                nc.gpsimd.tensor_tensor(out=abt[b][4][:], in0=abt[b][4][:],
                                        in1=abt[b][5][:], op=ALU.add)
            for b in range(BL):  # joins on DVE
                nc.vector.tensor_tensor(out=abt[b][0][:], in0=abt[b][0][:],
                                        in1=abt[b][2][:], op=ALU.add)
                nc.vector.tensor_tensor(out=abt[b][0][:], in0=abt[b][0][:],
                                        in1=abt[b][4][:], op=ALU.add)
            cb = [abt[b][0] for b in range(BL)]
            # u1 = cb*-2.5 + sig; negc = u1 + frac (in place over lgt)
            for b in range(BL):
                nc.vector.scalar_tensor_tensor(
                    out=lgt[b][:], in0=cb[b][:], scalar=-2.5, in1=lgt[b][:],
                    op0=ALU.mult, op1=ALU.add)
                nc.vector.tensor_tensor(out=lgt[b][:], in0=lgt[b][:],
                                        in1=frac[b][:], op=ALU.add)
            negc = lgt

            # per-partition top-8 + index -> cand columns
            candt = sm.tile([P, 2 * BL], F32, tag="candt")
            for b in range(BL):
                mx8 = sm.tile([P, 8], F32, tag=f"mx8_{b}", name=f"mx8_{b}")
                ix8 = sm.tile([P, 8], mybir.dt.uint32, tag=f"ix8_{b}",
                              name=f"ix8_{b}")
                nc.vector.max(out=mx8[:], in_=negc[b][:])
                nc.vector.max_index(out=ix8[:], in_max=mx8[:],
                                    in_values=negc[b][:])
                nc.vector.tensor_copy(out=candt[:, 2 * b : 2 * b + 1],
                                      in_=mx8[:, 0:1])
                ixf = sm.tile([P, 1], F32, tag=f"ixf{b}", name=f"ixf{b}")
                nc.vector.tensor_copy(out=ixf[:], in_=ix8[:, 0:1])
                nc.vector.tensor_scalar_add(out=candt[:, 2 * b + 1 : 2 * b + 2],
                                            in0=ixf[:], scalar1=col(b, 16))
            nc.sync.dma_start(out=cand[:], in_=candt[:])

    nc.finalize()
    return nc


def _prep_host(pred_logits, anchors, target_boxes, target_present):
    f32 = np.float32
    A = np.ascontiguousarray(anchors.reshape(O, QP, 6).astype(f32, copy=False))
    pad = lambda x: np.pad(x, ((0, 0), (0, NCH * N - QP)), mode="edge")

    comp = [pad(A[:, :, d]) for d in range(6)]  # [20, 8196] each
    rc = [np.maximum(comp[d], f32(0)) for d in range(3)]
    rsz = [np.maximum(comp[3 + d], f32(0)) for d in range(3)]
    alt = [rc[d] - f32(0.5) * rsz[d] for d in range(3)]
    arb = [rc[d] + f32(0.5) * rsz[d] for d in range(3)]
    vola = (rsz[0] * rsz[1]) * rsz[2]
    planes = alt + arb + rsz + [vola] + comp
    ath = np.stack([p.reshape(P, N) for p in planes])
    ath = np.ascontiguousarray(ath, dtype=f32)

    lgs = pred_logits.reshape(BS, O, QP).astype(f32, copy=False)
    lgs = np.pad(lgs, ((0, 0), (0, 0), (0, NCH * N - QP)), mode="edge")
    lg_all = lgs.reshape(BS, P, N)

    t = target_boxes.astype(f32, copy=False)          # [BS, O, 6]
    tc_, ts_ = t[..., :3], t[..., 3:]
    blt = tc_ - f32(0.5) * ts_
    brb = tc_ + f32(0.5) * ts_
    fd = brb - blt
    volb = (fd[..., 0] * fd[..., 1]) * fd[..., 2]
    prs = target_present.astype(f32, copy=False)      # [BS, O]

    in_maps = []
    for c in range(NCORES):
        b0 = c * BL
        lgc = np.ascontiguousarray(lg_all[b0 : b0 + BL], dtype=f32)
        scv = np.zeros((BL, P, 20), f32)
        sc3 = scv.reshape(BL, O, NCH, 20)
        for b in range(BL):
            gb = b0 + b
            sc3[b, :, :, 0:6] = -t[gb][:, None, :]
            sc3[b, :, :, 6:9] = blt[gb][:, None, :]
            sc3[b, :, :, 9:12] = brb[gb][:, None, :]
            sc3[b, :, :, 12:15] = fd[gb][:, None, :]
            sc3[b, :, :, 15] = volb[gb][:, None]
            sc3[b, :, :, 16] = np.arange(NCH, dtype=f32)[None, :] * f32(N)
            sc3[b, :, :, 17] = prs[gb][:, None] - f32(1)  # floor
        rwv = np.zeros((1, 512), f32)
        for b in range(BL):
            pr6 = np.repeat(prs[b0 + b], NCH)
            rwv[0, 256 * b : 256 * b + 120] = pr6
            rwv[0, 256 * b + 128 : 256 * b + 248] = pr6 - f32(1)
        in_maps.append({"ath": ath, "lg": lgc, "sc": scv, "rw": rwv})
    return in_maps


def kernel(pred_logits, pred_boxes, anchors, target_boxes, target_present,
           num_top_queries):
    k = int(num_top_queries)
    assert k == 1, f"kernel specialized for num_top_queries=1, got {k}"

    if "nc" not in _BUILT:
        _BUILT["nc"] = _build_nc()
    nc = _BUILT["nc"]

    pred_logits = np.asarray(pred_logits)
    anchors = np.asarray(anchors)
    target_boxes = np.asarray(target_boxes)
    target_present = np.asarray(target_present)
    in_maps = _prep_host(pred_logits, anchors, target_boxes, target_present)
    res = run_bass_kernel_spmd(nc, in_maps, core_ids=list(range(NCORES)))

    matches = np.zeros((BS, O, QP), np.int32)
    soft = np.empty((BS, O, QP), np.float32)
    present = target_present.astype(bool)
    for c, r in enumerate(res.results):
        b0 = c * BL
        soft[b0 : b0 + BL] = r["sout"].reshape(BL, O, NCH * N)[:, :, :QP]
        cd = r["cand"].reshape(O, NCH, 2 * BL)
        for b in range(BL):
            vals = cd[:, :, 2 * b]          # [O, NCH] chunk-max of negC/2
            gidx = cd[:, :, 2 * b + 1]      # [O, NCH] global q of chunk winner
            win = np.argmax(vals, axis=1)   # first max -> lowest chunk on ties
            for o in range(O):
                if present[b0 + b, o]:
                    matches[b0 + b, o, int(gidx[o, win[o]])] = 1
    return matches, soft
